# revision 33
# baseline (speedup 1.0000x reference)
"""Trainium2 Bass kernel for nn_Attention_48661979463892.

Multi-head attention: B=2, H=8, dk=dv=64, T=S=2048, E=512.
  keys    = Wk @ x[b]          -> per head [64, T]
  values  = Wv @ x[b]          -> per head [64, T]
  queries = Wq @ y[b]          -> per head [64, S]
  scores  = keys^T @ queries + mask            [T, S]
  attn    = softmax(0.125 * scores, axis=T)    (normalize over keys axis)
  out     = values @ attn                      [64, S]
  res     = W @ concat_heads(out) + b          -> [B, S, O]

Sharding: 16 (batch, head) pairs over 8 cores -> core c handles batch c//4,
head-pair c%4 (global head rows 128*(c%4) .. +128).  Each core emits a
partial [S, O] contribution of the final linear (its 128 v-channels); the
host sums 4 partials per batch and adds the bias.

v2 design notes (on top of the v1 software-pipelined structure):
  * x/y stream in 512-column chunks (all 4 contraction sub-blocks per
    chunk) so the keys/queries projections start ~3.5us in; the remaining
    projections are interleaved into the first s-chunk's t-loop.
  * The exp is the scalar-engine (ACT) bottleneck: (N+352)/1.2ns per
    tile, 73us for all 64 [128,1024] tiles.  A tunable subset of tiles is
    offloaded to the vector engine using a Schraudolph-style exponential:
    bf16_bits(exp(s*0.125)) ~= round(23.083*s + 16248.5), computed as ONE
    tensor_scalar (f32 PSUM -> int16 SBUF) and bitcast to bf16 for the AV
    matmul.  Softmax self-normalization cancels the common-mode error;
    the residual sawtooth (~1.8% rms on offloaded tiles) stays well under
    the 2e-2 budget.
  * Epilogue PSUM drains are split between ACT and DVE; result DMAs
    alternate between the sync and gpsimd queues.
"""

import numpy as np

N_CORES = 8
B, I, T, S, O = 2, 512, 2048, 2048, 512
H_PER_CORE = 2
DK = 64
SCALING = DK ** -0.5  # 0.125

MM_DTYPE = "bf16"
N_WARMUP_MM = 14

# Schraudolph exp -> bf16 bit trick: bits16 = round(A*score + B)
SCHRA_C = 7.5
SCHRA_MULT = (128.0 / float(np.log(2.0))) * SCALING
SCHRA_ADD = 127.0 * 128.0 - SCHRA_C
# which t-tiles of each 16-tile chunk use the DVE exp (rest use ACT)
# iterations whose h1-half exp runs on DVE (h0 always on ACT); excludes the
# chunk-boundary iterations where DVE is busy with osb casts / drains
DVE_TT = (2, 3, 4, 5, 6, 8, 9, 10, 11, 13, 14)

_BUILD_CACHE = {}


def _split_multi_waits(nc):
    """walrus in this toolchain accepts only ONE sync wait per instruction.
    Hoist extra waits onto same-engine NoOps inserted just before."""
    import concourse.mybir as mybir

    ctr = 0
    for fn in nc.m.functions:
        for blk in fn.blocks:
            new_insts = []
            for inst in blk.instructions:
                si = inst.sync_info
                if si is not None and len(si.on_wait) > 1:
                    waits = list(si.on_wait)
                    for w in waits[:-1]:
                        ctr += 1
                        nop = mybir.InstNoOp(
                            name=f"waitsplit-{ctr}", ins=[], outs=[]
                        )
                        nop.engine = inst.engine
                        nop.sync_info = mybir.SyncInfo(on_wait=[w], on_update=[])
                        new_insts.append(nop)
                    del si.on_wait[:-1]
                new_insts.append(inst)
            blk.instructions[:] = new_insts


def _build(with_mask):
    import concourse.bass as bass
    import concourse.mybir as mybir
    import concourse.tile as tile
    from concourse.bass import ts, ds

    f32 = mybir.dt.float32
    bf16 = mybir.dt.bfloat16
    i16 = mybir.dt.int16
    mmdt = bf16
    nc = bass.Bass()
    # x4/y4: [n_chunk, 128, j, 512] -- contiguous per (chunk, partition)
    x_p = nc.declare_dram_parameter("x4", [4, 128, 4, 512], mmdt, isOutput=False)
    y_p = nc.declare_dram_parameter("y4", [4, 128, 4, 512], mmdt, isOutput=False)
    wk_p = nc.declare_dram_parameter("wkT", [128, 4, 128], mmdt, isOutput=False)
    wv_p = nc.declare_dram_parameter("wvT", [128, 4, 128], mmdt, isOutput=False)
    wq_p = nc.declare_dram_parameter("wqT", [128, 4, 128], mmdt, isOutput=False)
    wc_p = nc.declare_dram_parameter("wcT", [2, 64, O], mmdt, isOutput=False)
    if with_mask:
        mask_p = nc.declare_dram_parameter("maskT", [16, 128, S], f32, isOutput=False)
    res_p = nc.declare_dram_parameter("res", [S, O], f32, isOutput=True)

    N_SC = S // 512    # s chunks of 512
    N_TT = T // 128    # t tiles of 128

    with tile.TileContext(nc) as tc:
        with (
            nc.allow_low_precision(reason="bf16 matmul operands / schraudolph exp"),
            tc.tile_pool(name="consts", bufs=1) as consts,
            tc.tile_pool(name="exps", bufs=4) as exps_pool,
            tc.tile_pool(name="epi", bufs=2) as epi_pool,
            tc.tile_pool(name="osb", bufs=4) as osb_pool,
            tc.tile_pool(name="resout", bufs=3) as res_pool,
            tc.tile_pool(name="ps_scores", bufs=2, space="PSUM") as ps_scores_pool,
            tc.tile_pool(name="ps_acc", bufs=2, space="PSUM") as ps_acc_pool,
            tc.tile_pool(name="ps_misc", bufs=2, space="PSUM") as ps_misc_pool,
        ):
            # dummy matmuls on scratch data keep the PE busy while the input
            # DMAs land, so the HAM clock-gate is warm when real work starts
            scratch_sb = consts.tile([128, 512], mmdt)
            nc.vector.memset(scratch_sb, 0.0)

            def filler_mm(n):
                for _ in range(n):
                    ps_w = ps_scores_pool.tile([128, 1024], f32, tag="ps_s", name="ps_w")
                    nc.tensor.matmul(
                        ps_w[:, 0:512], scratch_sb[:, 0:128], scratch_sb,
                        start=True, stop=True,
                    )

            filler_mm(N_WARMUP_MM)

            # ---------------- load inputs ----------------
            wk_sb = consts.tile([128, 4, 128], mmdt)
            wv_sb = consts.tile([128, 4, 128], mmdt)
            wq_sb = consts.tile([128, 4, 128], mmdt)
            wc_sb0 = consts.tile([64, O], mmdt)
            wc_sb1 = consts.tile([64, O], mmdt)
            # n-major SBUF layout: [128, n_chunk, j, 512] so each chunk's DMA
            # is a contiguous 4KB-per-partition burst
            x_sb = consts.tile([128, 4, 4, 512], mmdt)
            y_sb = consts.tile([128, 4, 4, 512], mmdt)

            # DMA plan.  One dma_start only sustains ~40-60 GB/s; a queue
            # reaches ~145 GB/s with ~4 in-flight instructions, and queued
            # DMAs share bandwidth fairly.  So: 4-way sub-split per chunk,
            # chunk n+1's issue gated behind chunk n's arrival by a dummy
            # dma that reads the landed chunk (data dep -> semaphore wait
            # on the issuing engine).
            gate_sb = consts.tile([1, 16], mmdt)

            def chunk4(eng, dst, src, n):
                for j in range(4):
                    eng.dma_start(out=dst[:, n, j], in_=src[n, :, j])

            def gate(eng, dep_ap):
                # reads 4 elems of each j-sub-DMA so it depends on all four
                eng.dma_start(out=gate_sb, in_=dep_ap[0:1, :, 0:4])

            # sync: x chunks, gated serially
            chunk4(nc.sync, x_sb, x_p, 0)
            gate(nc.sync, x_sb[:, 0])
            chunk4(nc.sync, x_sb, x_p, 1)
            gate(nc.sync, x_sb[:, 1])
            chunk4(nc.sync, x_sb, x_p, 2)
            gate(nc.sync, x_sb[:, 2])
            chunk4(nc.sync, x_sb, x_p, 3)
            # gpsimd: weights + y; y0 split gpsimd/scalar.  The scalar-side
            # y0 halves are UNgated (issue-and-forget), so they cannot block
            # the ACT exp stream; all gated DMAs live on sync/gpsimd.
            nc.gpsimd.dma_start(out=wk_sb, in_=wk_p[:, :, :])
            nc.gpsimd.dma_start(out=wq_sb, in_=wq_p[:, :, :])
            nc.gpsimd.dma_start(out=y_sb[:, 0, 0], in_=y_p[0, :, 0])
            nc.gpsimd.dma_start(out=y_sb[:, 0, 1], in_=y_p[0, :, 1])
            nc.scalar.dma_start(out=y_sb[:, 0, 2], in_=y_p[0, :, 2])
            nc.scalar.dma_start(out=y_sb[:, 0, 3], in_=y_p[0, :, 3])
            nc.gpsimd.dma_start(out=wv_sb, in_=wv_p[:, :, :])
            nc.gpsimd.dma_start(out=wc_sb0, in_=wc_p[0])
            nc.gpsimd.dma_start(out=wc_sb1, in_=wc_p[1])
            gate(nc.gpsimd, y_sb[:, 0])
            chunk4(nc.gpsimd, y_sb, y_p, 1)
            chunk4(nc.gpsimd, y_sb, y_p, 2)
            chunk4(nc.gpsimd, y_sb, y_p, 3)

            # ---------------- projections ----------------
            keys_sb = consts.tile([128, T], mmdt)
            qs_sb = consts.tile([128, S], mmdt)

            def proj_group(dst, w_sb, src, n, drain_eng):
                """project one 512-wide n-slice (all 4 contraction chunks)."""
                ps = ps_misc_pool.tile([128, 512], f32, tag="misc", name="pjp")
                for j in range(4):
                    nc.tensor.matmul(
                        ps,
                        w_sb[:, j, :],
                        src[:, n, j, :],
                        start=(j == 0),
                        stop=(j == 3),
                    )
                if drain_eng == "act":
                    nc.scalar.copy(dst[:, ts(n, 512)], ps)
                else:
                    nc.vector.tensor_copy(out=dst[:, ts(n, 512)], in_=ps)

            # values^T with ones columns: [t_part=128, tt, 130]
            # cols 0:64 head0, col 64 ones, cols 65:129 head1, col 129 ones.
            valT_sb = consts.tile([128, N_TT, 130], mmdt)
            nc.vector.memset(valT_sb[:, :, 64:65], 1.0)
            nc.vector.memset(valT_sb[:, :, 129:130], 1.0)

            def valT_proj(tt):
                ps = ps_misc_pool.tile([128, 128], f32, tag="misc", name="vps")
                for j in range(4):
                    nc.tensor.matmul(
                        ps,
                        x_sb[:, tt // 4, j, ts(tt % 4, 128)],
                        wv_sb[:, j, :],
                        start=(j == 0),
                        stop=(j == 3),
                    )
                eng = nc.scalar if tt % 2 == 0 else nc.vector
                if tt % 2 == 0:
                    nc.scalar.copy(valT_sb[:, tt, 0:64], ps[:, 0:64])
                    nc.scalar.copy(valT_sb[:, tt, 65:129], ps[:, 64:128])
                else:
                    nc.vector.tensor_copy(out=valT_sb[:, tt, 0:64], in_=ps[:, 0:64])
                    nc.vector.tensor_copy(out=valT_sb[:, tt, 65:129], in_=ps[:, 64:128])

            # pre-loop: only the first chunk of keys and queries
            proj_group(keys_sb, wk_sb, x_sb, 0, "dve")
            filler_mm(2)
            proj_group(qs_sb, wq_sb, y_sb, 0, "act")

            # (sc, tt) -> list of extra projection work emitted that iteration
            extra_sched = {
                (0, 0): [("valT", 0), ("valT", 1)],
                (0, 1): [("keys", 1), ("valT", 2)],
                (0, 2): [("valT", 3)],
                (0, 3): [("valT", 4)],
                (0, 4): [("keys", 2), ("valT", 5)],
                (0, 5): [("valT", 6)],
                (0, 6): [("valT", 7)],
                (0, 7): [("valT", 8)],
                (0, 8): [("keys", 3), ("valT", 9)],
                (0, 9): [("valT", 10)],
                (0, 10): [("valT", 11)],
                (0, 11): [("qs", 1), ("valT", 12)],
                (0, 12): [("valT", 13)],
                (0, 13): [("valT", 14)],
                (0, 14): [("valT", 15)],
                (1, 2): [("qs", 2)],
                (2, 2): [("qs", 3)],
            }

            def extra_work(sc, tt):
                for kind, n in extra_sched.get((sc, tt), ()):
                    if kind == "valT":
                        valT_proj(n)
                    elif kind == "keys":
                        proj_group(keys_sb, wk_sb, x_sb, n, "dve")
                    else:
                        proj_group(qs_sb, wq_sb, y_sb, n, "act")

            # ---------------- attention main loop (software-pipelined) ----
            # epilogue st-groups of chunk sc-1 are emitted inside chunk sc's
            # t-loop (at EPI_TT) so only the last chunk's epilogue trails
            # must avoid DVE_TT iterations: the drains would head-block the
            # DVE exp stream
            EPI_TT = {2: 0, 5: 1, 8: 2, 11: 3}

            def t_loop(sc, prev_osb):
                """scores + exp + AV accumulation for s chunk `sc`; the
                previous chunk's normalize is issued after the first tile so
                its latency hides inside this chunk's stream."""
                rec_prev = None
                ps_o = [
                    ps_acc_pool.tile([65, 512], f32, tag="av", name=f"ps_o{h}")
                    for h in range(2)
                ]
                pend = None

                def emit_av(p):
                    p_tt, p_ex0, p_ex1 = p
                    for h, p_ex in ((0, p_ex0), (1, p_ex1)):
                        nc.tensor.matmul(
                            ps_o[h],
                            valT_sb[:, p_tt, 65 * h : 65 * h + 65],
                            p_ex,
                            start=(p_tt == 0),
                            stop=(p_tt == N_TT - 1),
                        )

                for tt in range(N_TT):
                    extra_work(sc, tt)
                    if tt == 1 and prev_osb is not None:
                        rec_prev = normalize(prev_osb)
                    ps_s = ps_scores_pool.tile([128, 1024], f32, tag="ps_s", name="ps_s")
                    if with_mask:
                        m_sb = exps_pool.tile([128, 512], f32, tag="mask", name="m_sb")
                        nc.sync.dma_start(out=m_sb, in_=mask_p[tt][:, ts(sc, 512)])
                    for h in range(2):
                        nc.tensor.matmul(
                            ps_s[:, ts(h, 512)],
                            keys_sb[64 * h : 64 * h + 64, ts(tt, 128)],
                            qs_sb[64 * h : 64 * h + 64, ts(sc, 512)],
                            start=True,
                            stop=True,
                        )
                        if with_mask:
                            nc.vector.tensor_tensor(
                                ps_s[:, ts(h, 512)],
                                ps_s[:, ts(h, 512)],
                                m_sb,
                                mybir.AluOpType.add,
                            )
                    # consumers split per head: ACT exps h0 while DVE
                    # schraudolphs h1 -- both [128,512], concurrent, so the
                    # slot-recycle chain sees ~720ns instead of ~1150ns
                    exa = exps_pool.tile([128, 1024], mmdt, tag="exa", name="exa")
                    nc.scalar.activation(
                        out=exa[:, 0:512],
                        in_=ps_s[:, 0:512],
                        func=mybir.ActivationFunctionType.Exp,
                        scale=float(SCALING),
                    )
                    if (not with_mask) and (tt in DVE_TT):
                        exd = exps_pool.tile([128, 512], i16, tag="exd", name="exd")
                        nc.vector.tensor_scalar(
                            exd,
                            ps_s[:, 512:1024],
                            float(SCHRA_MULT),
                            float(SCHRA_ADD),
                            mybir.AluOpType.mult,
                            mybir.AluOpType.add,
                        )
                        ex1 = exd.bitcast(bf16)
                    else:
                        nc.scalar.activation(
                            out=exa[:, 512:1024],
                            in_=ps_s[:, 512:1024],
                            func=mybir.ActivationFunctionType.Exp,
                            scale=float(SCALING),
                        )
                        ex1 = exa[:, 512:1024]
                    # AV deferred one iteration: scores(tt+1) lands in the
                    # PE queue ahead of AV(tt), so the slot pipeline isn't
                    # serialized behind the consumer->AV chain
                    if pend is not None:
                        emit_av(pend)
                    pend = (tt, exa[:, 0:512], ex1)
                emit_av(pend)
                osb = []
                for h in range(2):
                    o_un = osb_pool.tile([65, 512], mmdt, tag=f"osb{h}", name=f"osb{h}")
                    # both on DVE: an ACT-side cast here would head-block the
                    # next chunk's exp stream behind the last-AV wait
                    nc.vector.tensor_copy(out=o_un, in_=ps_o[h])
                    osb.append(o_un)
                return osb, rec_prev

            def normalize(osb):
                """1/colsum as per-partition columns: transpose each [1,128]
                colsum slice into a PSUM column via a K=1 matmul, then one
                tiny [128, 8] reciprocal.  Column h*4+st holds head h,
                s-subtile st."""
                cs_ps = ps_misc_pool.tile([128, 8], f32, tag="misc", name="cs_ps")
                one_mm = valT_sb[64:65, 0, 64:65]
                for h in range(2):
                    for st in range(4):
                        nc.tensor.matmul(
                            cs_ps[:, h * 4 + st : h * 4 + st + 1],
                            osb[h][64:65, ts(st, 128)],
                            one_mm,
                            start=True,
                            stop=True,
                        )
                rec_col = epi_pool.tile([128, 8], f32, tag="rec", name="rec_col")
                nc.vector.reciprocal(out=rec_col, in_=cs_ps)
                return rec_col

            def epilogue_st(sc, osb, rec_col, st, last=False):
                """one s-subtile of the final linear + fused 1/colsum drain."""
                if last and st < 2:
                    pr = ps_scores_pool.tile(
                        [128, 1024], f32, tag="ps_s", name="ps_rp"
                    )
                    ps_r0 = pr[:, 0:512]
                    ps_r1 = pr[:, 512:1024]
                else:
                    ps_r0 = ps_misc_pool.tile(
                        [128, 512], f32, tag="misc", name="ps_r0"
                    )
                    ps_r1 = ps_misc_pool.tile(
                        [128, 512], f32, tag="misc", name="ps_r1"
                    )
                nc.tensor.matmul(
                    ps_r0, osb[0][0:64, ts(st, 128)], wc_sb0,
                    start=True, stop=True,
                )
                nc.tensor.matmul(
                    ps_r1, osb[1][0:64, ts(st, 128)], wc_sb1,
                    start=True, stop=True,
                )
                a_sb = res_pool.tile([128, O], f32, tag="a_sb", name="a_sb")
                if last:
                    nc.scalar.activation(
                        out=a_sb,
                        in_=ps_r0,
                        func=mybir.ActivationFunctionType.Copy,
                        scale=rec_col[:, st : st + 1],
                    )
                else:
                    nc.vector.tensor_scalar_mul(
                        a_sb, ps_r0, rec_col[:, st : st + 1]
                    )
                r_sb = res_pool.tile([128, O], f32)
                nc.vector.scalar_tensor_tensor(
                    out=r_sb,
                    in0=ps_r1,
                    scalar=rec_col[:, 4 + st : 5 + st],
                    in1=a_sb,
                    op0=mybir.AluOpType.mult,
                    op1=mybir.AluOpType.add,
                )
                rows = ds(sc * 512 + st * 128, 128)
                nc.sync.dma_start(out=res_p[rows, 0:256], in_=r_sb[:, 0:256])
                nc.gpsimd.dma_start(out=res_p[rows, 256:512], in_=r_sb[:, 256:512])

            prev_osb = None
            for sc in range(N_SC):
                osb, rec_prev = t_loop(sc, prev_osb)
                if rec_prev is not None:
                    for st in range(4):
                        epilogue_st(sc - 1, prev_osb, rec_prev, st)
                prev_osb = osb
            rec_last = normalize(prev_osb)
            for st in range(4):
                epilogue_st(N_SC - 1, prev_osb, rec_last, st, last=True)

    _split_multi_waits(nc)
    return nc


def _get_nc(with_mask):
    key = (with_mask, MM_DTYPE)
    if key not in _BUILD_CACHE:
        _BUILD_CACHE[key] = _build(with_mask)
    return _BUILD_CACHE[key]


def _mm_np_dtype():
    import ml_dtypes
    return np.dtype(ml_dtypes.bfloat16)


def _make_in_maps(x, y, mask, Wk, Wv, Wq, W, with_mask):
    mdt = _mm_np_dtype()
    in_maps = []
    for c in range(N_CORES):
        bb, hp = divmod(c, 4)
        e_sl = slice(128 * hp, 128 * hp + 128)
        im = {
            "x4": np.ascontiguousarray(
                x[bb].reshape(4, 128, 4, 512).transpose(2, 1, 0, 3).astype(mdt)
            ),
            "y4": np.ascontiguousarray(
                y[bb].reshape(4, 128, 4, 512).transpose(2, 1, 0, 3).astype(mdt)
            ),
            "wkT": np.ascontiguousarray(
                Wk[e_sl].T.reshape(4, 128, 128).transpose(1, 0, 2).astype(mdt)
            ),
            "wvT": np.ascontiguousarray(
                Wv[e_sl].T.reshape(4, 128, 128).transpose(1, 0, 2).astype(mdt)
            ),
            "wqT": np.ascontiguousarray(
                Wq[e_sl].T.reshape(4, 128, 128).transpose(1, 0, 2).astype(mdt)
            ),
            "wcT": np.ascontiguousarray(
                np.stack(
                    [
                        W[:, 128 * hp : 128 * hp + 64].T,
                        W[:, 128 * hp + 64 : 128 * hp + 128].T,
                    ]
                ).astype(mdt)
            ),
        }
        if with_mask:
            im["maskT"] = np.ascontiguousarray(mask.reshape(16, 128, S))
        in_maps.append(im)
    return in_maps


def kernel(x, y, mask, Wk, Wv, Wq, W, b):
    from concourse.bass_utils import run_bass_kernel_spmd

    x = np.asarray(x, dtype=np.float32)
    y = np.asarray(y, dtype=np.float32)
    mask = np.asarray(mask, dtype=np.float32)
    Wk = np.asarray(Wk, dtype=np.float32)
    Wv = np.asarray(Wv, dtype=np.float32)
    Wq = np.asarray(Wq, dtype=np.float32)
    W = np.asarray(W, dtype=np.float32)
    b = np.asarray(b, dtype=np.float32)

    with_mask = bool(np.any(mask))
    nc = _get_nc(with_mask)
    in_maps = _make_in_maps(x, y, mask, Wk, Wv, Wq, W, with_mask)

    r = run_bass_kernel_spmd(nc, in_maps, core_ids=list(range(N_CORES)))
    parts = [r.results[c]["res"] for c in range(N_CORES)]
    out = np.stack(
        [
            parts[0] + parts[1] + parts[2] + parts[3],
            parts[4] + parts[5] + parts[6] + parts[7],
        ],
        axis=0,
    )
    out += b[None, None, :]
    return out.astype(np.float32)


# revision 42
# speedup vs baseline: 1.0894x; 1.0894x over previous
"""Trainium2 Bass kernel for nn_Attention_48661979463892.

Multi-head attention: B=2, H=8, dk=dv=64, T=S=2048, E=512.
  keys    = Wk @ x[b]          -> per head [64, T]
  values  = Wv @ x[b]          -> per head [64, T]
  queries = Wq @ y[b]          -> per head [64, S]
  scores  = keys^T @ queries + mask            [T, S]
  attn    = softmax(0.125 * scores, axis=T)    (normalize over keys axis)
  out     = values @ attn                      [64, S]
  res     = W @ concat_heads(out) + b          -> [B, S, O]

Sharding: 16 (batch, head) pairs over 8 cores -> core c handles batch c//4,
head-pair c%4 (global head rows 128*(c%4) .. +128).  Each core emits a
partial [S, O] contribution of the final linear (its 128 v-channels); the
host sums 4 partials per batch and adds the bias.

v2 design notes (on top of the v1 software-pipelined structure):
  * x/y stream in 512-column chunks (all 4 contraction sub-blocks per
    chunk) so the keys/queries projections start ~3.5us in; the remaining
    projections are interleaved into the first s-chunk's t-loop.
  * The exp is the scalar-engine (ACT) bottleneck: (N+352)/1.2ns per
    tile, 73us for all 64 [128,1024] tiles.  A tunable subset of tiles is
    offloaded to the vector engine using a Schraudolph-style exponential:
    bf16_bits(exp(s*0.125)) ~= round(23.083*s + 16248.5), computed as ONE
    tensor_scalar (f32 PSUM -> int16 SBUF) and bitcast to bf16 for the AV
    matmul.  Softmax self-normalization cancels the common-mode error;
    the residual sawtooth (~1.8% rms on offloaded tiles) stays well under
    the 2e-2 budget.
  * Epilogue PSUM drains are split between ACT and DVE; result DMAs
    alternate between the sync and gpsimd queues.
"""

import numpy as np

N_CORES = 8
B, I, T, S, O = 2, 512, 2048, 2048, 512
H_PER_CORE = 2
DK = 64
SCALING = DK ** -0.5  # 0.125

MM_DTYPE = "bf16"
N_WARMUP_MM = 10

# Schraudolph exp -> bf16 bit trick: bits16 = round(A*score + B)
SCHRA_C = 7.5
SCHRA_MULT = (128.0 / float(np.log(2.0))) * SCALING
SCHRA_ADD = 127.0 * 128.0 - SCHRA_C
# which t-tiles of each 16-tile chunk use the DVE exp (rest use ACT)
# which t-tiles of each 16-tile chunk use the DVE exp (rest use ACT)
DVE_TT = (3, 6, 9, 12, 14)

_BUILD_CACHE = {}


def _split_multi_waits(nc):
    """walrus in this toolchain accepts only ONE sync wait per instruction.
    Hoist extra waits onto same-engine NoOps inserted just before."""
    import concourse.mybir as mybir

    ctr = 0
    for fn in nc.m.functions:
        for blk in fn.blocks:
            new_insts = []
            for inst in blk.instructions:
                si = inst.sync_info
                if si is not None and len(si.on_wait) > 1:
                    waits = list(si.on_wait)
                    for w in waits[:-1]:
                        ctr += 1
                        nop = mybir.InstNoOp(
                            name=f"waitsplit-{ctr}", ins=[], outs=[]
                        )
                        nop.engine = inst.engine
                        nop.sync_info = mybir.SyncInfo(on_wait=[w], on_update=[])
                        new_insts.append(nop)
                    del si.on_wait[:-1]
                new_insts.append(inst)
            blk.instructions[:] = new_insts


def _build(with_mask):
    import concourse.bass as bass
    import concourse.mybir as mybir
    import concourse.tile as tile
    from concourse.bass import ts, ds

    f32 = mybir.dt.float32
    bf16 = mybir.dt.bfloat16
    i16 = mybir.dt.int16
    mmdt = bf16
    nc = bass.Bass()
    # x4/y4: [n_chunk, 128, j, 512] -- contiguous per (chunk, partition)
    x_p = nc.declare_dram_parameter("x4", [4, 128, 4, 512], mmdt, isOutput=False)
    y_p = nc.declare_dram_parameter("y4", [4, 128, 4, 512], mmdt, isOutput=False)
    wk_p = nc.declare_dram_parameter("wkT", [128, 4, 128], mmdt, isOutput=False)
    wv_p = nc.declare_dram_parameter("wvT", [128, 4, 128], mmdt, isOutput=False)
    wq_p = nc.declare_dram_parameter("wqT", [128, 4, 128], mmdt, isOutput=False)
    wc_p = nc.declare_dram_parameter("wcT", [2, 64, O], mmdt, isOutput=False)
    if with_mask:
        mask_p = nc.declare_dram_parameter("maskT", [16, 128, S], f32, isOutput=False)
    res_p = nc.declare_dram_parameter("res", [S, O], f32, isOutput=True)

    N_SC = S // 512    # s chunks of 512
    N_TT = T // 128    # t tiles of 128

    with tile.TileContext(nc) as tc:
        with (
            nc.allow_low_precision(reason="bf16 matmul operands / schraudolph exp"),
            tc.tile_pool(name="consts", bufs=1) as consts,
            tc.tile_pool(name="exps", bufs=4) as exps_pool,
            tc.tile_pool(name="epi", bufs=2) as epi_pool,
            tc.tile_pool(name="osb", bufs=4) as osb_pool,
            tc.tile_pool(name="resout", bufs=3) as res_pool,
            tc.tile_pool(name="ps_scores", bufs=2, space="PSUM") as ps_scores_pool,
            tc.tile_pool(name="ps_acc", bufs=2, space="PSUM") as ps_acc_pool,
            tc.tile_pool(name="ps_misc", bufs=2, space="PSUM") as ps_misc_pool,
        ):
            # dummy matmuls on scratch data keep the PE busy while the input
            # DMAs land, so the HAM clock-gate is warm when real work starts
            scratch_sb = consts.tile([128, 512], mmdt)
            nc.vector.memset(scratch_sb, 0.0)

            def filler_mm(n):
                for _ in range(n):
                    ps_w = ps_scores_pool.tile([128, 1024], f32, tag="ps_s", name="ps_w")
                    nc.tensor.matmul(
                        ps_w[:, 0:512], scratch_sb[:, 0:128], scratch_sb,
                        start=True, stop=True,
                    )

            filler_mm(N_WARMUP_MM)

            # ---------------- load inputs ----------------
            wk_sb = consts.tile([128, 4, 128], mmdt)
            wv_sb = consts.tile([128, 4, 128], mmdt)
            wq_sb = consts.tile([128, 4, 128], mmdt)
            wc_sb0 = consts.tile([64, O], mmdt)
            wc_sb1 = consts.tile([64, O], mmdt)
            # n-major SBUF layout: [128, n_chunk, j, 512] so each chunk's DMA
            # is a contiguous 4KB-per-partition burst
            x_sb = consts.tile([128, 4, 4, 512], mmdt)
            y_sb = consts.tile([128, 4, 4, 512], mmdt)

            # DMA plan.  One dma_start only sustains ~40-60 GB/s; a queue
            # reaches ~145 GB/s with ~4 in-flight instructions, and queued
            # DMAs share bandwidth fairly.  So: 4-way sub-split per chunk,
            # chunk n+1's issue gated behind chunk n's arrival by a dummy
            # dma that reads the landed chunk (data dep -> semaphore wait
            # on the issuing engine).
            gate_sb = consts.tile([1, 16], mmdt)

            def chunk4(eng, dst, src, n):
                for j in range(4):
                    eng.dma_start(out=dst[:, n, j], in_=src[n, :, j])

            def gate(eng, dep_ap):
                # reads 4 elems of each j-sub-DMA so it depends on all four
                eng.dma_start(out=gate_sb, in_=dep_ap[0:1, :, 0:4])

            # sync: x chunks, gated serially
            chunk4(nc.sync, x_sb, x_p, 0)
            gate(nc.sync, x_sb[:, 0])
            chunk4(nc.sync, x_sb, x_p, 1)
            gate(nc.sync, x_sb[:, 1])
            chunk4(nc.sync, x_sb, x_p, 2)
            gate(nc.sync, x_sb[:, 2])
            chunk4(nc.sync, x_sb, x_p, 3)
            # gpsimd: weights + y; y0 split gpsimd/scalar.  The scalar-side
            # y0 halves are UNgated (issue-and-forget), so they cannot block
            # the ACT exp stream; all gated DMAs live on sync/gpsimd.
            nc.gpsimd.dma_start(out=wk_sb, in_=wk_p[:, :, :])
            nc.gpsimd.dma_start(out=wq_sb, in_=wq_p[:, :, :])
            nc.gpsimd.dma_start(out=y_sb[:, 0, 0], in_=y_p[0, :, 0])
            nc.gpsimd.dma_start(out=y_sb[:, 0, 1], in_=y_p[0, :, 1])
            nc.scalar.dma_start(out=y_sb[:, 0, 2], in_=y_p[0, :, 2])
            nc.scalar.dma_start(out=y_sb[:, 0, 3], in_=y_p[0, :, 3])
            nc.gpsimd.dma_start(out=wv_sb, in_=wv_p[:, :, :])
            nc.gpsimd.dma_start(out=wc_sb0, in_=wc_p[0])
            nc.gpsimd.dma_start(out=wc_sb1, in_=wc_p[1])
            gate(nc.gpsimd, y_sb[:, 0])
            chunk4(nc.gpsimd, y_sb, y_p, 1)
            chunk4(nc.gpsimd, y_sb, y_p, 2)
            chunk4(nc.gpsimd, y_sb, y_p, 3)

            # ---------------- projections ----------------
            keys_sb = consts.tile([128, T], mmdt)
            qs_sb = consts.tile([128, S], mmdt)

            def proj_group(dst, w_sb, src, n, drain_eng):
                """project one 512-wide n-slice (all 4 contraction chunks)."""
                ps = ps_misc_pool.tile([128, 512], f32, tag="misc", name="pjp")
                for j in range(4):
                    nc.tensor.matmul(
                        ps,
                        w_sb[:, j, :],
                        src[:, n, j, :],
                        start=(j == 0),
                        stop=(j == 3),
                    )
                if drain_eng == "act":
                    nc.scalar.copy(dst[:, ts(n, 512)], ps)
                else:
                    nc.vector.tensor_copy(out=dst[:, ts(n, 512)], in_=ps)

            # values^T with ones columns: [t_part=128, tt, 130]
            # cols 0:64 head0, col 64 ones, cols 65:129 head1, col 129 ones.
            valT_sb = consts.tile([128, N_TT, 130], mmdt)
            nc.vector.memset(valT_sb[:, :, 64:65], 1.0)
            nc.vector.memset(valT_sb[:, :, 129:130], 1.0)

            def valT_proj(tt):
                ps = ps_misc_pool.tile([128, 128], f32, tag="misc", name="vps")
                for j in range(4):
                    nc.tensor.matmul(
                        ps,
                        x_sb[:, tt // 4, j, ts(tt % 4, 128)],
                        wv_sb[:, j, :],
                        start=(j == 0),
                        stop=(j == 3),
                    )
                eng = nc.scalar if tt % 2 == 0 else nc.vector
                if tt % 2 == 0:
                    nc.scalar.copy(valT_sb[:, tt, 0:64], ps[:, 0:64])
                    nc.scalar.copy(valT_sb[:, tt, 65:129], ps[:, 64:128])
                else:
                    nc.vector.tensor_copy(out=valT_sb[:, tt, 0:64], in_=ps[:, 0:64])
                    nc.vector.tensor_copy(out=valT_sb[:, tt, 65:129], in_=ps[:, 64:128])

            # pre-loop: only the first chunk of keys and queries
            proj_group(keys_sb, wk_sb, x_sb, 0, "dve")
            filler_mm(2)
            proj_group(qs_sb, wq_sb, y_sb, 0, "act")

            # (sc, tt) -> list of extra projection work emitted that iteration
            extra_sched = {
                (0, 0): [("valT", 0), ("valT", 1)],
                (0, 1): [("keys", 1), ("valT", 2)],
                (0, 2): [("valT", 3)],
                (0, 3): [("valT", 4)],
                (0, 4): [("keys", 2), ("valT", 5)],
                (0, 5): [("valT", 6)],
                (0, 6): [("valT", 7)],
                (0, 7): [("valT", 8)],
                (0, 8): [("keys", 3), ("valT", 9)],
                (0, 9): [("valT", 10)],
                (0, 10): [("valT", 11)],
                (0, 11): [("qs", 1), ("valT", 12)],
                (0, 12): [("valT", 13)],
                (0, 13): [("valT", 14)],
                (0, 14): [("valT", 15)],
                (1, 2): [("qs", 2)],
                (2, 2): [("qs", 3)],
            }

            def extra_work(sc, tt):
                for kind, n in extra_sched.get((sc, tt), ()):
                    if kind == "valT":
                        valT_proj(n)
                    elif kind == "keys":
                        proj_group(keys_sb, wk_sb, x_sb, n, "dve")
                    else:
                        proj_group(qs_sb, wq_sb, y_sb, n, "act")

            # ---------------- attention main loop (software-pipelined) ----
            # epilogue st-groups of chunk sc-1 are emitted inside chunk sc's
            # t-loop (at EPI_TT) so only the last chunk's epilogue trails
            # must avoid DVE_TT iterations: the drains would head-block the
            # DVE exp stream
            EPI_TT = {2: 0, 5: 1, 8: 2, 11: 3}

            def t_loop(sc, prev_osb):
                """scores + exp + AV accumulation for s chunk `sc`; the
                previous chunk's normalize is issued after the first tile so
                its latency hides inside this chunk's stream."""
                rec_prev = None
                ps_o = [
                    ps_acc_pool.tile([65, 512], f32, tag="av", name=f"ps_o{h}")
                    for h in range(2)
                ]
                pend = None

                def emit_av(p):
                    p_tt, p_ex0, p_ex1 = p
                    for h, p_ex in ((0, p_ex0), (1, p_ex1)):
                        nc.tensor.matmul(
                            ps_o[h],
                            valT_sb[:, p_tt, 65 * h : 65 * h + 65],
                            p_ex,
                            start=(p_tt == 0),
                            stop=(p_tt == N_TT - 1),
                        )

                for tt in range(N_TT):
                    extra_work(sc, tt)
                    if tt == 1 and prev_osb is not None:
                        rec_prev = normalize(prev_osb)
                    ps_s = ps_scores_pool.tile([128, 1024], f32, tag="ps_s", name="ps_s")
                    if with_mask:
                        m_sb = exps_pool.tile([128, 512], f32, tag="mask", name="m_sb")
                        nc.sync.dma_start(out=m_sb, in_=mask_p[tt][:, ts(sc, 512)])
                    for h in range(2):
                        nc.tensor.matmul(
                            ps_s[:, ts(h, 512)],
                            keys_sb[64 * h : 64 * h + 64, ts(tt, 128)],
                            qs_sb[64 * h : 64 * h + 64, ts(sc, 512)],
                            start=True,
                            stop=True,
                        )
                        if with_mask:
                            nc.vector.tensor_tensor(
                                ps_s[:, ts(h, 512)],
                                ps_s[:, ts(h, 512)],
                                m_sb,
                                mybir.AluOpType.add,
                            )
                    if (not with_mask) and (tt in DVE_TT):
                        exd = exps_pool.tile([128, 1024], i16, tag="exd", name="exd")
                        nc.vector.tensor_scalar(
                            exd,
                            ps_s,
                            float(SCHRA_MULT),
                            float(SCHRA_ADD),
                            mybir.AluOpType.mult,
                            mybir.AluOpType.add,
                        )
                        ex = exd.bitcast(bf16)
                    else:
                        exa = exps_pool.tile([128, 1024], mmdt, tag="exa", name="exa")
                        nc.scalar.activation(
                            out=exa,
                            in_=ps_s,
                            func=mybir.ActivationFunctionType.Exp,
                            scale=float(SCALING),
                        )
                        ex = exa
                    # AV deferred one iteration: scores(tt+1) lands in the
                    # PE queue ahead of AV(tt), so the slot pipeline isn't
                    # serialized behind the consumer->AV chain
                    if pend is not None:
                        emit_av(pend)
                    pend = (tt, ex[:, 0:512], ex[:, 512:1024])
                emit_av(pend)
                osb = []
                for h in range(2):
                    o_un = osb_pool.tile([65, 512], mmdt, tag=f"osb{h}", name=f"osb{h}")
                    if h == 0:
                        nc.vector.tensor_copy(out=o_un, in_=ps_o[h])
                    else:
                        nc.scalar.copy(o_un, ps_o[h])
                    osb.append(o_un)
                return osb, rec_prev

            def normalize(osb):
                """1/colsum as per-partition columns: transpose each [1,128]
                colsum slice into a PSUM column via a K=1 matmul, then one
                tiny [128, 8] reciprocal.  Column h*4+st holds head h,
                s-subtile st."""
                cs_ps = ps_misc_pool.tile([128, 8], f32, tag="misc", name="cs_ps")
                one_mm = valT_sb[64:65, 0, 64:65]
                for h in range(2):
                    for st in range(4):
                        nc.tensor.matmul(
                            cs_ps[:, h * 4 + st : h * 4 + st + 1],
                            osb[h][64:65, ts(st, 128)],
                            one_mm,
                            start=True,
                            stop=True,
                        )
                rec_col = epi_pool.tile([128, 8], f32, tag="rec", name="rec_col")
                nc.vector.reciprocal(out=rec_col, in_=cs_ps)
                return rec_col

            def epilogue_st(sc, osb, rec_col, st, last=False):
                """one s-subtile of the final linear + fused 1/colsum drain."""
                if last and st < 2:
                    pr = ps_scores_pool.tile(
                        [128, 1024], f32, tag="ps_s", name="ps_rp"
                    )
                    ps_r0 = pr[:, 0:512]
                    ps_r1 = pr[:, 512:1024]
                else:
                    ps_r0 = ps_misc_pool.tile(
                        [128, 512], f32, tag="misc", name="ps_r0"
                    )
                    ps_r1 = ps_misc_pool.tile(
                        [128, 512], f32, tag="misc", name="ps_r1"
                    )
                nc.tensor.matmul(
                    ps_r0, osb[0][0:64, ts(st, 128)], wc_sb0,
                    start=True, stop=True,
                )
                nc.tensor.matmul(
                    ps_r1, osb[1][0:64, ts(st, 128)], wc_sb1,
                    start=True, stop=True,
                )
                a_sb = res_pool.tile([128, O], f32, tag="a_sb", name="a_sb")
                if last:
                    nc.scalar.activation(
                        out=a_sb,
                        in_=ps_r0,
                        func=mybir.ActivationFunctionType.Copy,
                        scale=rec_col[:, st : st + 1],
                    )
                else:
                    nc.vector.tensor_scalar_mul(
                        a_sb, ps_r0, rec_col[:, st : st + 1]
                    )
                r_sb = res_pool.tile([128, O], f32)
                nc.vector.scalar_tensor_tensor(
                    out=r_sb,
                    in0=ps_r1,
                    scalar=rec_col[:, 4 + st : 5 + st],
                    in1=a_sb,
                    op0=mybir.AluOpType.mult,
                    op1=mybir.AluOpType.add,
                )
                rows = ds(sc * 512 + st * 128, 128)
                if last:
                    nc.sync.dma_start(out=res_p[rows, 0:128], in_=r_sb[:, 0:128])
                    nc.gpsimd.dma_start(out=res_p[rows, 128:256], in_=r_sb[:, 128:256])
                    nc.sync.dma_start(out=res_p[rows, 256:384], in_=r_sb[:, 256:384])
                    nc.gpsimd.dma_start(out=res_p[rows, 384:512], in_=r_sb[:, 384:512])
                else:
                    nc.sync.dma_start(out=res_p[rows, 0:256], in_=r_sb[:, 0:256])
                    nc.gpsimd.dma_start(out=res_p[rows, 256:512], in_=r_sb[:, 256:512])

            prev_osb = None
            for sc in range(N_SC):
                osb, rec_prev = t_loop(sc, prev_osb)
                if rec_prev is not None:
                    for st in range(4):
                        epilogue_st(sc - 1, prev_osb, rec_prev, st)
                prev_osb = osb
            rec_last = normalize(prev_osb)
            for st in range(4):
                epilogue_st(N_SC - 1, prev_osb, rec_last, st, last=True)

    _split_multi_waits(nc)
    return nc


def _get_nc(with_mask):
    key = (with_mask, MM_DTYPE)
    if key not in _BUILD_CACHE:
        _BUILD_CACHE[key] = _build(with_mask)
    return _BUILD_CACHE[key]


def _mm_np_dtype():
    import ml_dtypes
    return np.dtype(ml_dtypes.bfloat16)


def _make_in_maps(x, y, mask, Wk, Wv, Wq, W, with_mask):
    mdt = _mm_np_dtype()
    in_maps = []
    for c in range(N_CORES):
        bb, hp = divmod(c, 4)
        e_sl = slice(128 * hp, 128 * hp + 128)
        im = {
            "x4": np.ascontiguousarray(
                x[bb].reshape(4, 128, 4, 512).transpose(2, 1, 0, 3).astype(mdt)
            ),
            "y4": np.ascontiguousarray(
                y[bb].reshape(4, 128, 4, 512).transpose(2, 1, 0, 3).astype(mdt)
            ),
            "wkT": np.ascontiguousarray(
                Wk[e_sl].T.reshape(4, 128, 128).transpose(1, 0, 2).astype(mdt)
            ),
            "wvT": np.ascontiguousarray(
                Wv[e_sl].T.reshape(4, 128, 128).transpose(1, 0, 2).astype(mdt)
            ),
            "wqT": np.ascontiguousarray(
                Wq[e_sl].T.reshape(4, 128, 128).transpose(1, 0, 2).astype(mdt)
            ),
            "wcT": np.ascontiguousarray(
                np.stack(
                    [
                        W[:, 128 * hp : 128 * hp + 64].T,
                        W[:, 128 * hp + 64 : 128 * hp + 128].T,
                    ]
                ).astype(mdt)
            ),
        }
        if with_mask:
            im["maskT"] = np.ascontiguousarray(mask.reshape(16, 128, S))
        in_maps.append(im)
    return in_maps


def kernel(x, y, mask, Wk, Wv, Wq, W, b):
    from concourse.bass_utils import run_bass_kernel_spmd

    x = np.asarray(x, dtype=np.float32)
    y = np.asarray(y, dtype=np.float32)
    mask = np.asarray(mask, dtype=np.float32)
    Wk = np.asarray(Wk, dtype=np.float32)
    Wv = np.asarray(Wv, dtype=np.float32)
    Wq = np.asarray(Wq, dtype=np.float32)
    W = np.asarray(W, dtype=np.float32)
    b = np.asarray(b, dtype=np.float32)

    with_mask = bool(np.any(mask))
    nc = _get_nc(with_mask)
    in_maps = _make_in_maps(x, y, mask, Wk, Wv, Wq, W, with_mask)

    r = run_bass_kernel_spmd(nc, in_maps, core_ids=list(range(N_CORES)))
    parts = [r.results[c]["res"] for c in range(N_CORES)]
    out = np.stack(
        [
            parts[0] + parts[1] + parts[2] + parts[3],
            parts[4] + parts[5] + parts[6] + parts[7],
        ],
        axis=0,
    )
    out += b[None, None, :]
    return out.astype(np.float32)
